# revision 13
# baseline (speedup 1.0000x reference)
"""ConvAttention Trainium2 kernel (v6).

Per-core (data-parallel over batch, 8 cores, 1 image each):
  q/k/v = depthwise 3x3 conv over x [56,56,64], then full attention over
  N=3136 tokens with softmax(q.k * 8), then ctx @ Wp + bp.

Layout strategy (see v5 notes):
  - x staged on host as two [128, 58, 58] images (xpT row-shift pair, x3
    col-shift pair) in f32r; convs are tap-stacked K=128 matmuls (5 per
    conv tile); Wp/bv/bp fold into the v-conv; bk dropped (cancels).
  - exp on ACT from PSUM in alternating 3/2-chunk groups; AV uses pT
    chunks as lhsT into [token, embed] accumulators with a ones column
    for the softmax denominator (reciprocal+mul normalization on DVE).

v6 changes (cost-model driven):
  - DMA plan rebuilt for the serial DMA-engine model: all loads on the
    sync queue in demand order (stq, bqb1, xpT rows 0-9, stkv, x3 rows
    0-9, then alternating 10-row xpT/x3 chunks), so the conv/QK chain
    for tile 0 unblocks ~4.5us earlier and kv tiles stream just in time.
  - PE p-state warmup starts from a DVE-memset f32r tile (no Pool
    dependency) so the clock is at full rate when the first conv lands.
  - v_nat transposes no longer sit between kv-conv 0 and QK group 0 in
    PE program order; they drain opportunistically inside tile 0's
    groups (emit_vnat).
  - Last q-tile's AV no longer serializes after the final exp: subs 0/1
    accumulate chunk-wise one group behind exp in the (idle) conv psum
    slots, subs 2/3 catch up during the last group in the freed psSa /
    psC slots, so only ~2us of AV+norm+DMA tail remains.
"""

import sys

import numpy as np

if "/opt/trn_rl_repo" not in sys.path:
    sys.path.insert(0, "/opt/trn_rl_repo")

H = 56
W = 56
C = 64
E = 64
N = H * W               # 3136 tokens
HP = H + 2              # padded
WP = W + 2
NQ = 448                # q-tile (8 spatial rows)
NQT = N // NQ           # 7
KC = 128                # k-chunk (partition dim of s^T tiles)
NKC = (N + KC - 1) // KC  # 25 (last chunk is 64 real tokens)
NPAD = NKC * KC         # 3200 (k padded with zeros)
NCORES = 8

# exp chunk-groups per tile: alternating 3/2 so the two score pools fit in
# 5 PSUM banks total while still double-buffering QK against exp
GRP_SIZES = [3, 2, 3, 2, 3, 2, 3, 2, 3, 2]
GRP_OFF = [0, 3, 5, 8, 10, 13, 15, 18, 20, 23]
NGRP = len(GRP_SIZES)
# kv-conv tile that must be complete before QK of group g (any q-tile)
KV_NEED = [min(((GRP_OFF[g] + GRP_SIZES[g]) * KC - 1) // NQ, NQT - 1)
           for g in range(NGRP)]
# stacked conv slots: (lower tap, upper tap or None); taps t = 3*i + j.
# Slots 0-2 pair rows 0+1 via xpT's row-shifted upper half; slot 3 pairs
# (2,0)+(2,1) via x3's col-shifted upper half; slot 4 is the single (2,2).
CONV_SLOTS = [(0, 3), (1, 4), (2, 5), (6, 7), (8, None)]
# emission order: the single K=64 tap first (fewest dependencies)
SLOT_ORDER = (4, 0, 1, 2, 3)

_CACHE = {}


def _grp_of(kc):
    for g in range(NGRP):
        if GRP_OFF[g] <= kc < GRP_OFF[g] + GRP_SIZES[g]:
            return g
    raise ValueError(kc)


def _prep_x(xi):
    """Host staging: [56,56,64] -> (xpT, x3) [128, HP, WP] float32."""
    base = np.zeros((C, HP, WP), np.float32)
    base[:, 1:1 + H, 1:1 + W] = np.ascontiguousarray(xi.transpose(2, 0, 1))
    xp = np.zeros((128, HP, WP), np.float32)
    xp[0:C] = base
    xp[C:128, 0:HP - 1] = base[:, 1:HP]
    x3 = np.zeros((128, HP, WP), np.float32)
    x3[0:C] = base
    x3[C:128, :, 0:WP - 1] = base[:, :, 1:WP]
    return xp, x3


def _prep_weights(wq, wk, wv, bq, bv, Wp, bp):
    """Host staging of the tap-stacked conv lhsT blocks and biases.

    stq [128, 5, 64]: diag(wq[lt]) on rows 0-63, diag(wq[ut]) on 64-127.
    stkv [128, 5, 128]: cols 0-63 diag(wk), cols 64-127 diag(wv) @ Wp.
    bqb1 [128, 2]: col 0 rows 0-63 = bq; col 1 rows 64-127 = bv@Wp + bp.
    """
    wq = np.asarray(wq, np.float32).reshape(9, C)
    wk = np.asarray(wk, np.float32).reshape(9, C)
    wv = np.asarray(wv, np.float32).reshape(9, C)
    Wp = np.asarray(Wp, np.float32)
    eye = np.eye(C, dtype=np.float32)
    stq = np.zeros((128, 5, C), np.float32)
    stkv = np.zeros((128, 5, 128), np.float32)
    for s, (lt, ut) in enumerate(CONV_SLOTS):
        stq[0:C, s, :] = eye * wq[lt][:, None]
        stkv[0:C, s, 0:C] = eye * wk[lt][:, None]
        stkv[0:C, s, C:128] = wv[lt][:, None] * Wp
        if ut is not None:
            stq[C:128, s, :] = eye * wq[ut][:, None]
            stkv[C:128, s, 0:C] = eye * wk[ut][:, None]
            stkv[C:128, s, C:128] = wv[ut][:, None] * Wp
    bqb1 = np.zeros((128, 2), np.float32)
    bqb1[0:C, 0] = np.asarray(bq, np.float32)
    bqb1[C:128, 1] = np.asarray(bv, np.float32) @ Wp + np.asarray(bp, np.float32)
    return stq, stkv, bqb1


def _build(level=99):
    import concourse.bacc as bacc
    import concourse.tile as tile
    from concourse import mybir
    from concourse.masks import make_identity
    from concourse.tile import add_dep_helper

    F32 = mybir.dt.float32
    F32R = mybir.dt.float32r
    BF16 = mybir.dt.bfloat16
    AF = mybir.ActivationFunctionType

    nc = bacc.Bacc(None, target_bir_lowering=False, debug=False)

    x_d = nc.dram_tensor("x", [128, HP, WP], F32R, kind="ExternalInput")
    x3_d = nc.dram_tensor("x3", [128, HP, WP], F32R, kind="ExternalInput")
    stq_d = nc.dram_tensor("stq", [128, 5, C], F32R, kind="ExternalInput")
    stkv_d = nc.dram_tensor("stkv", [128, 5, 128], F32R, kind="ExternalInput")
    bqb1_d = nc.dram_tensor("bqb1", [128, 2], F32, kind="ExternalInput")
    # per-tile padded output: token qt*448 + s*128 + p -> out_d[qt, s*128+p]
    # (s=3 rows 64-127 are never written; host slices them off)
    out_d = nc.dram_tensor("out", [NQT, 4 * 128, E], F32, kind="ExternalOutput")

    # row-chunked image loads, sized for the serial DMA model: rows 0-9 for
    # tile 0's convs up front, then uniform 8-row chunks just ahead of the
    # kv-conv demand curve. x3 rows 0-1 are never read (conv slot 3 reads
    # rows 8ct+2..8ct+9), so its first chunk starts at row 2.
    RCH = [10, 18, 26, 34, 42, 50, HP]

    with tile.TileContext(nc) as tc:
        with tc.tile_pool(name="const", bufs=1) as const, \
             tc.tile_pool(name="big", bufs=1) as big:
            # warmup tile first: DVE memset -> PE p-state ramp starts ~1us in
            wtile = const.tile([128, 128], F32R)
            nc.vector.memset(wtile[:], 0.0)

            xpT = big.tile([128, HP, WP], F32R)
            x3 = big.tile([128, HP, WP], F32R)
            st_q = const.tile([128, 5, C], F32R)
            st_kv = const.tile([128, 5, 128], F32R)
            bqb1 = const.tile([128, 2], F32)

            # DMA plan: single (sync) queue; the shared DMA engine device is
            # serial in the cost model, so order == priority. Front: q-conv
            # deps, then kv deps, then alternating 8-row chunks.
            nc.sync.dma_start(st_q[:], stq_d[:])
            nc.sync.dma_start(bqb1[:], bqb1_d[:])
            nc.sync.dma_start(xpT[:, 0:RCH[0], :], x_d[:, 0:RCH[0], :])
            nc.sync.dma_start(st_kv[:], stkv_d[:])
            nc.sync.dma_start(x3[:, 2:RCH[0], :], x3_d[:, 2:RCH[0], :])
            for ci in range(len(RCH) - 1):
                r0, r1 = RCH[ci], RCH[ci + 1]
                nc.sync.dma_start(xpT[:, r0:r1, :], x_d[:, r0:r1, :])
                nc.sync.dma_start(x3[:, r0:r1, :], x3_d[:, r0:r1, :])

            ident_f = const.tile([128, 128], F32)
            make_identity(nc, ident_f[:])
            ident_b = const.tile([128, 128], BF16)
            nc.vector.tensor_copy(ident_b[:], ident_f[:])

            zsc = const.tile([128, 128], F32)
            nc.vector.memset(zsc[:], 0.0)
            ones_f = const.tile([128, NKC], F32)
            nc.vector.memset(ones_f[:], 1.0)

            qT = big.tile([C, N], F32R)            # q^T  [c, token]
            kT = big.tile([C, NPAD], F32R)         # k^T  [c, token], zero pad
            vT = big.tile([128, N], BF16)          # v''^T on partitions 64-127
            v_nat = big.tile([128, NKC, C + 1], BF16)  # [tok%128, chunk, e|1]

            nc.vector.tensor_copy(kT[:, N:NPAD], zsc[0:C, 0:NPAD - N])
            nc.vector.tensor_copy(v_nat[:, :, C], ones_f[:])

            with tc.tile_pool(name="ps2", bufs=2, space="PSUM") as ps2, \
                 tc.tile_pool(name="psSa", bufs=1, space="PSUM") as psSa, \
                 tc.tile_pool(name="psSb", bufs=1, space="PSUM") as psSb, \
                 tc.tile_pool(name="psC", bufs=1, space="PSUM") as psC, \
                 tc.tile_pool(name="sbP", bufs=2 * NGRP) as sbP, \
                 tc.tile_pool(name="sbO", bufs=4) as sbO, \
                 tc.tile_pool(name="sbI", bufs=4) as sbI:

                # PE warmup: set pe_busy_start early so the clock ramps to
                # full rate during the image DMA wait
                ptw = ps2.tile([128, NQ], F32, name="ptw", tag="cv")
                for _ in range(16):
                    nc.tensor.matmul(ptw[0:128, 0:128], wtile[:, 0:128],
                                     wtile[:, 0:128], start=True, stop=True)

                # ---- incremental emitters ----------------------------------
                st = {"kv": 0, "vn": 0, "pctx": None}

                def conv_matmuls(pdst, lhsT, ct, mwid, slots=range(5)):
                    r0 = ct * 8
                    first = None
                    for i in slots:
                        s = SLOT_ORDER[i]
                        if s < 3:
                            rhs = xpT[:, r0:r0 + 8, s:s + W]
                            lh = lhsT[:, s, 0:mwid]
                        elif s == 3:
                            rhs = x3[:, r0 + 2:r0 + 10, 0:W]
                            lh = lhsT[:, s, 0:mwid]
                        else:
                            rhs = xpT[0:C, r0 + 2:r0 + 10, 2:2 + W]
                            lh = lhsT[0:C, s, 0:mwid]
                        mm = nc.tensor.matmul(pdst[:], lh, rhs,
                                              start=(i == 0), stop=(i == 4))
                        if first is None:
                            first = mm
                    return first

                def emit_vt_add(pkv, ct):
                    nc.vector.tensor_scalar_add(
                        vT[C:128, ct * NQ:(ct + 1) * NQ], pkv[C:128, :],
                        bqb1[C:128, 1:2])

                def emit_kv(upto):
                    while st["kv"] <= min(upto, NQT - 1):
                        ct = st["kv"]
                        pkv = ps2.tile([128, NQ], F32, name="pkv", tag="cv")
                        first = conv_matmuls(pkv, st_kv, ct, 128)
                        if st.get("qk_dep") is not None:
                            # keep the tile scheduler from hoisting kv convs
                            # ahead of the pending QK group on the PE
                            add_dep_helper(first.ins, st["qk_dep"].ins,
                                           reason="kv conv after QK group")
                        nc.vector.tensor_copy(kT[:, ct * NQ:(ct + 1) * NQ],
                                              pkv[0:C, :])
                        emit_vt_add(pkv, ct)
                        st["kv"] += 1

                def emit_vnat():
                    # v_nat transposes, batched 4 chunks per PSUM tile /
                    # copy to amortize the DVE PSUM-access overhead
                    top = st["kv"] * NQ
                    while st["vn"] < NKC:
                        kc0 = st["vn"]
                        nb = min(4, NKC - kc0)
                        end = kc0 + nb - 1
                        cw_last = min(KC, N - end * KC)
                        if end * KC + cw_last > top:
                            break
                        tp = psC.tile([128, 4, C], BF16, name="tpv",
                                      tag="ctx")
                        for j in range(nb):
                            kc = kc0 + j
                            cw = min(KC, N - kc * KC)
                            nc.tensor.transpose(
                                tp[0:cw, j, :],
                                vT[C:128, kc * KC:kc * KC + cw],
                                ident_b[C:128, C:128])
                        cw = min(KC, N - (kc0 + nb - 1) * KC)
                        if nb == 4 and cw == KC:
                            nc.vector.tensor_copy(
                                v_nat[:, kc0:kc0 + nb, 0:C], tp[:, 0:nb, :])
                        else:
                            for j in range(nb):
                                kc = kc0 + j
                                cw = min(KC, N - kc * KC)
                                nc.vector.tensor_copy(
                                    v_nat[0:cw, kc, 0:C], tp[0:cw, j, :])
                        st["vn"] += nb

                def emit_qconv_slots(pq, qt, slots):
                    r0 = qt * 8
                    for i in slots:
                        s = SLOT_ORDER[i]
                        if s < 3:
                            rhs = xpT[:, r0:r0 + 8, s:s + W]
                            lh = st_q[:, s, :]
                        elif s == 3:
                            rhs = x3[:, r0 + 2:r0 + 10, 0:W]
                            lh = st_q[:, s, :]
                        else:
                            rhs = xpT[0:C, r0 + 2:r0 + 10, 2:2 + W]
                            lh = st_q[0:C, s, :]
                        nc.tensor.matmul(pq[:], lh, rhs,
                                         start=(i == 0), stop=(i == 4))

                def emit_qcopy(pq, qt):
                    if qt == 0:
                        # ACT is idle before the first exp; DVE is busy with
                        # lhsT builds
                        nc.scalar.add(qT[:, qt * NQ:(qt + 1) * NQ], pq[:],
                                      bqb1[0:C, 0:1])
                    else:
                        nc.vector.tensor_scalar_add(
                            qT[:, qt * NQ:(qt + 1) * NQ], pq[:], bqb1[0:C, 0:1])

                def emit_av_chunks(pT_tiles, s, pctx, chunks):
                    # chunk-matmuls of one q-subtile into accumulator pctx
                    s0 = s * 128
                    sw = min(128, NQ - s0)
                    for kc in chunks:
                        g = _grp_of(kc)
                        j = kc - GRP_OFF[g]
                        cw = 64 if kc == NKC - 1 else 128
                        nc.tensor.matmul(
                            pctx[0:sw, :],
                            pT_tiles[g][0:cw, j, s0:s0 + sw],
                            v_nat[0:cw, kc, :],
                            start=(kc == 0), stop=(kc == NKC - 1))

                def emit_av_batch(pT_tiles, s, half):
                    # 25 chunk-matmuls of one q-subtile, split in two halves;
                    # one pending psum group at a time (zero-region rule)
                    if half == 0:
                        st["pctx"] = psC.tile([128, C + 1], F32,
                                              name="pctx", tag="ctx")
                    chunks = range(0, 13) if half == 0 else range(13, NKC)
                    emit_av_chunks(pT_tiles, s, st["pctx"], chunks)

                def emit_norm_sub(pctx, qt, s):
                    # normalize into the tile's gathered [128, 4, E] buffer;
                    # sub 3 triggers the single per-tile store DMA
                    s0 = s * 128
                    sw = min(128, NQ - s0)
                    inv = sbI.tile([128, 1], F32, name="inv", tag="inv")
                    nc.vector.reciprocal(inv[0:sw, :], pctx[0:sw, C:C + 1])
                    if s == 0:
                        st["osb4"] = sbO.tile([128, 4, E], F32, name="osb4",
                                              tag="out")
                    osb4 = st["osb4"]
                    nc.vector.tensor_scalar_mul(
                        osb4[0:sw, s, :], pctx[0:sw, 0:C], inv[0:sw, 0:1])
                    if s == 3:
                        nc.sync.dma_start(
                            out_d[qt, :, :].rearrange("(s p) e -> p s e", p=128),
                            osb4[:, :, :])

                def flush_prev(prev, g):
                    # AV batches one group later than minimal so the psC WAR
                    # (sub start vs previous norm) never stalls the PE
                    if prev is None or g < 1 or g > 8:
                        return
                    qt_prev, pT_tiles = prev
                    emit_av_batch(pT_tiles, (g - 1) // 2, (g - 1) % 2)
                    if (g - 1) % 2 == 1:
                        emit_norm_sub(st["pctx"], qt_prev, (g - 1) // 2)

                # ---- lead-in: q-conv(0) + kv(0) ----------------------------
                # PE order tuned for the kT critical path: xpT-only slots of
                # both convs first, then kv's x3 slot (so the kT copy can
                # start ASAP), then q's x3 slot + bias copy. vT add deferred
                # past QK g0 (engine-counter waits would otherwise put it on
                # QK's critical path).
                tap_sched = {0: (0,), 1: (1,), 2: (2,), 3: (3,), 4: (4,)}
                if level >= 2:
                    pq = ps2.tile([C, NQ], F32, name="pq", tag="cv")
                    emit_qconv_slots(pq, 0, range(4))
                    pkv0 = ps2.tile([128, NQ], F32, name="pkv", tag="cv")
                    conv_matmuls(pkv0, st_kv, 0, 128, slots=range(4))
                    conv_matmuls(pkv0, st_kv, 0, 128, slots=(4,))
                    nc.vector.tensor_copy(kT[:, 0:NQ], pkv0[0:C, :])
                    emit_qconv_slots(pq, 0, (4,))
                    emit_qcopy(pq, 0)
                    st["kv"] = 1
                    st["pkv0"] = pkv0

                prev = None
                last = NQT - 1
                for qt in range(NQT if level >= 5 else 0):
                    q0 = qt * NQ
                    pq_next = None
                    pT_tiles = []
                    pc6 = [None, None, None, None]  # last-tile accumulators
                    if qt == last and level >= 6:
                        pc6[0] = ps2.tile([128, C + 1], F32, name="pc6a",
                                          tag="cv")
                        pc6[1] = ps2.tile([128, C + 1], F32, name="pc6b",
                                          tag="cv")
                    for g in range(NGRP):
                        gsz = GRP_SIZES[g]
                        pool = psSa if g % 2 == 0 else psSb
                        ps_s = pool.tile([128, gsz, 512], F32, name="ps_s",
                                         tag="sa" if g % 2 == 0 else "sb")
                        for j in range(gsz):
                            kc = GRP_OFF[g] + j
                            qk_mm = nc.tensor.matmul(
                                ps_s[:, j, 0:NQ],
                                kT[:, kc * KC:(kc + 1) * KC],
                                qT[:, q0:q0 + NQ],
                                start=True, stop=True)
                        st["qk_dep"] = qk_mm
                        flush_prev(prev, g)
                        if qt == last and level >= 6:
                            if g == 8:
                                # tile-5 AV fully flushed; psC free for sub 3
                                pc6[3] = psC.tile([128, C + 1], F32,
                                                  name="pc6d", tag="ctx")
                            if g >= 1:
                                lo = GRP_OFF[g - 1]
                                hi = lo + GRP_SIZES[g - 1]
                                emit_av_chunks(pT_tiles, 0, pc6[0],
                                               range(lo, hi))
                                emit_av_chunks(pT_tiles, 1, pc6[1],
                                               range(lo, hi))
                            if g == NGRP - 1:
                                # catch-up: subs 2/3 over groups 0..8
                                hi = GRP_OFF[g]
                                emit_av_chunks(pT_tiles, 2, pc6[2],
                                               range(0, hi))
                                emit_av_chunks(pT_tiles, 3, pc6[3],
                                               range(0, hi))
                        if qt == 0:
                            # kv-conv tiles + v_nat stream in under tile 0
                            if g == 0:
                                emit_vt_add(st["pkv0"], 0)
                            if g + 1 < NGRP:
                                emit_kv(KV_NEED[g + 1])
                                emit_vnat()
                            elif qt + 1 < NQT:
                                emit_vnat()
                                pq_next = ps2.tile([C, NQ], F32,
                                                   name="pq", tag="cv")
                                emit_qconv_slots(pq_next, 1, range(5))
                                emit_qcopy(pq_next, 1)
                        elif qt + 1 < NQT:
                            if g in tap_sched:
                                if pq_next is None:
                                    pq_next = ps2.tile([C, NQ], F32,
                                                       name="pq", tag="cv")
                                emit_qconv_slots(pq_next, qt + 1, tap_sched[g])
                            if g == 5:
                                emit_qcopy(pq_next, qt + 1)
                        pTt = sbP.tile([128, 3, NQ], BF16, name="pTt", tag="p")
                        nc.scalar.activation(
                            pTt[:, 0:gsz, :], ps_s[:, 0:gsz, 0:NQ],
                            AF.Exp, scale=8.0)
                        pT_tiles.append(pTt)
                        if qt == last and level >= 6 and g == 8:
                            # psSa's last score group read; slot free for sub 2
                            pc6[2] = psSa.tile([128, C + 1], F32,
                                               name="pc6c", tag="sa")
                    if level >= 6:
                        if qt < last:
                            prev = (qt, pT_tiles)
                        else:
                            # tail: last group's chunks for all subs, then
                            # norm + store per sub
                            lo = GRP_OFF[NGRP - 1]
                            hi = lo + GRP_SIZES[NGRP - 1]
                            for s in range(4):
                                emit_av_chunks(pT_tiles, s, pc6[s],
                                               range(lo, hi))
                            for s in range(4):
                                emit_norm_sub(pc6[s], qt, s)

    nc.compile()
    return nc


def _get_nc():
    if "nc" not in _CACHE:
        _CACHE["nc"] = _build()
    return _CACHE["nc"]


def kernel(x, wq, bq, wk, bk, wv, bv, Wp, bp):
    from concourse.bass_utils import run_bass_kernel_spmd

    nc = _get_nc()
    x = np.asarray(x, dtype=np.float32)
    stq, stkv, bqb1 = _prep_weights(wq, wk, wv, bq, bv, Wp, bp)
    shared = {"stq": stq, "stkv": stkv, "bqb1": bqb1}
    in_maps = []
    for i in range(NCORES):
        xp, x3 = _prep_x(x[i])
        in_maps.append(dict(shared, x=xp, x3=x3))
    res = run_bass_kernel_spmd(nc, in_maps, core_ids=list(range(NCORES)))
    out = np.stack([
        np.asarray(res.results[i]["out"])
        .reshape(NQT, 4 * 128, E)[:, 0:NQ, :].reshape(H, W, E)
        for i in range(NCORES)
    ])
    return out


# revision 14
# speedup vs baseline: 1.0271x; 1.0271x over previous
"""ConvAttention Trainium2 kernel (v6).

Per-core (data-parallel over batch, 8 cores, 1 image each):
  q/k/v = depthwise 3x3 conv over x [56,56,64], then full attention over
  N=3136 tokens with softmax(q.k * 8), then ctx @ Wp + bp.

Layout strategy (see v5 notes):
  - x staged on host as two [128, 58, 58] images (xpT row-shift pair, x3
    col-shift pair) in f32r; convs are tap-stacked K=128 matmuls (5 per
    conv tile); Wp/bv/bp fold into the v-conv; bk dropped (cancels).
  - exp on ACT from PSUM in alternating 3/2-chunk groups; AV uses pT
    chunks as lhsT into [token, embed] accumulators with a ones column
    for the softmax denominator (reciprocal+mul normalization on DVE).

v6 changes (cost-model driven):
  - DMA plan rebuilt for the serial DMA-engine model: all loads on the
    sync queue in demand order (stq, bqb1, xpT rows 0-9, stkv, x3 rows
    0-9, then alternating 10-row xpT/x3 chunks), so the conv/QK chain
    for tile 0 unblocks ~4.5us earlier and kv tiles stream just in time.
  - PE p-state warmup starts from a DVE-memset f32r tile (no Pool
    dependency) so the clock is at full rate when the first conv lands.
  - v_nat transposes no longer sit between kv-conv 0 and QK group 0 in
    PE program order; they drain opportunistically inside tile 0's
    groups (emit_vnat).
  - Last q-tile's AV no longer serializes after the final exp: subs 0/1
    accumulate chunk-wise one group behind exp in the (idle) conv psum
    slots, subs 2/3 catch up during the last group in the freed psSa /
    psC slots, so only ~2us of AV+norm+DMA tail remains.
"""

import sys

import numpy as np

if "/opt/trn_rl_repo" not in sys.path:
    sys.path.insert(0, "/opt/trn_rl_repo")

H = 56
W = 56
C = 64
E = 64
N = H * W               # 3136 tokens
HP = H + 2              # padded
WP = W + 2
NQ = 448                # q-tile (8 spatial rows)
NQT = N // NQ           # 7
KC = 128                # k-chunk (partition dim of s^T tiles)
NKC = (N + KC - 1) // KC  # 25 (last chunk is 64 real tokens)
NPAD = NKC * KC         # 3200 (k padded with zeros)
NCORES = 8

# exp chunk-groups per tile: alternating 3/2 so the two score pools fit in
# 5 PSUM banks total while still double-buffering QK against exp
GRP_SIZES = [3, 2, 3, 2, 3, 2, 3, 2, 3, 2]
GRP_OFF = [0, 3, 5, 8, 10, 13, 15, 18, 20, 23]
NGRP = len(GRP_SIZES)
# kv-conv tile that must be complete before QK of group g (any q-tile)
KV_NEED = [min(((GRP_OFF[g] + GRP_SIZES[g]) * KC - 1) // NQ, NQT - 1)
           for g in range(NGRP)]
# stacked conv slots: (lower tap, upper tap or None); taps t = 3*i + j.
# Slots 0-2 pair rows 0+1 via xpT's row-shifted upper half; slot 3 pairs
# (2,0)+(2,1) via x3's col-shifted upper half; slot 4 is the single (2,2).
CONV_SLOTS = [(0, 3), (1, 4), (2, 5), (6, 7), (8, None)]
# emission order: the single K=64 tap first (fewest dependencies)
SLOT_ORDER = (4, 0, 1, 2, 3)

_CACHE = {}


def _grp_of(kc):
    for g in range(NGRP):
        if GRP_OFF[g] <= kc < GRP_OFF[g] + GRP_SIZES[g]:
            return g
    raise ValueError(kc)


def _prep_x(xi):
    """Host staging: [56,56,64] -> (xpT, x3) [128, HP, WP] float32."""
    base = np.zeros((C, HP, WP), np.float32)
    base[:, 1:1 + H, 1:1 + W] = np.ascontiguousarray(xi.transpose(2, 0, 1))
    xp = np.zeros((128, HP, WP), np.float32)
    xp[0:C] = base
    xp[C:128, 0:HP - 1] = base[:, 1:HP]
    x3 = np.zeros((128, HP, WP), np.float32)
    x3[0:C] = base
    x3[C:128, :, 0:WP - 1] = base[:, :, 1:WP]
    return xp, x3


def _prep_weights(wq, wk, wv, bq, bv, Wp, bp):
    """Host staging of the tap-stacked conv lhsT blocks and biases.

    stq [128, 5, 64]: diag(wq[lt]) on rows 0-63, diag(wq[ut]) on 64-127.
    stkv [128, 5, 128]: cols 0-63 diag(wk), cols 64-127 diag(wv) @ Wp.
    bqb1 [128, 2]: col 0 rows 0-63 = bq; col 1 rows 64-127 = bv@Wp + bp.
    """
    wq = np.asarray(wq, np.float32).reshape(9, C)
    wk = np.asarray(wk, np.float32).reshape(9, C)
    wv = np.asarray(wv, np.float32).reshape(9, C)
    Wp = np.asarray(Wp, np.float32)
    eye = np.eye(C, dtype=np.float32)
    stq = np.zeros((128, 5, C), np.float32)
    stkv = np.zeros((128, 5, 128), np.float32)
    for s, (lt, ut) in enumerate(CONV_SLOTS):
        stq[0:C, s, :] = eye * wq[lt][:, None]
        stkv[0:C, s, 0:C] = eye * wk[lt][:, None]
        stkv[0:C, s, C:128] = wv[lt][:, None] * Wp
        if ut is not None:
            stq[C:128, s, :] = eye * wq[ut][:, None]
            stkv[C:128, s, 0:C] = eye * wk[ut][:, None]
            stkv[C:128, s, C:128] = wv[ut][:, None] * Wp
    bqb1 = np.zeros((128, 2), np.float32)
    bqb1[0:C, 0] = np.asarray(bq, np.float32)
    bqb1[C:128, 1] = np.asarray(bv, np.float32) @ Wp + np.asarray(bp, np.float32)
    return stq, stkv, bqb1


def _build(level=99):
    import concourse.bacc as bacc
    import concourse.tile as tile
    from concourse import mybir
    from concourse.masks import make_identity
    from concourse.tile import add_dep_helper

    F32 = mybir.dt.float32
    F32R = mybir.dt.float32r
    BF16 = mybir.dt.bfloat16
    AF = mybir.ActivationFunctionType

    nc = bacc.Bacc(None, target_bir_lowering=False, debug=False)

    x_d = nc.dram_tensor("x", [128, HP, WP], F32R, kind="ExternalInput")
    x3_d = nc.dram_tensor("x3", [128, HP, WP], F32R, kind="ExternalInput")
    stq_d = nc.dram_tensor("stq", [128, 5, C], F32R, kind="ExternalInput")
    stkv_d = nc.dram_tensor("stkv", [128, 5, 128], F32R, kind="ExternalInput")
    bqb1_d = nc.dram_tensor("bqb1", [128, 2], F32, kind="ExternalInput")
    # per-tile padded output: token qt*448 + s*128 + p -> out_d[qt, s*128+p]
    # (s=3 rows 64-127 are never written; host slices them off)
    out_d = nc.dram_tensor("out", [NQT, 4 * 128, E], F32, kind="ExternalOutput")

    # row-chunked image loads, sized for the serial DMA model: rows 0-9 for
    # tile 0's convs up front, then uniform 8-row chunks just ahead of the
    # kv-conv demand curve. x3 rows 0-1 are never read (conv slot 3 reads
    # rows 8ct+2..8ct+9), so its first chunk starts at row 2.
    RCH = [10, 18, 26, 34, 42, 50, HP]

    with tile.TileContext(nc) as tc:
        with tc.tile_pool(name="const", bufs=1) as const, \
             tc.tile_pool(name="big", bufs=1) as big:
            # warmup tile first: DVE memset -> PE p-state ramp starts ~1us in
            wtile = const.tile([128, 128], F32R)
            nc.vector.memset(wtile[:], 0.0)

            xpT = big.tile([128, HP, WP], F32R)
            x3 = big.tile([128, HP, WP], F32R)
            st_q = const.tile([128, 5, C], F32R)
            st_kv = const.tile([128, 5, 128], F32R)
            bqb1 = const.tile([128, 2], F32)

            # DMA plan: single (sync) queue; the shared DMA engine device is
            # serial in the cost model, so order == priority. Front: q-conv
            # deps, then kv deps, then alternating 8-row chunks.
            nc.sync.dma_start(st_q[:], stq_d[:])
            nc.sync.dma_start(bqb1[:], bqb1_d[:])
            nc.sync.dma_start(xpT[:, 0:RCH[0], :], x_d[:, 0:RCH[0], :])
            nc.sync.dma_start(st_kv[:], stkv_d[:])
            nc.sync.dma_start(x3[:, 2:RCH[0], :], x3_d[:, 2:RCH[0], :])
            for ci in range(len(RCH) - 1):
                r0, r1 = RCH[ci], RCH[ci + 1]
                nc.sync.dma_start(xpT[:, r0:r1, :], x_d[:, r0:r1, :])
                nc.sync.dma_start(x3[:, r0:r1, :], x3_d[:, r0:r1, :])

            ident_f = const.tile([128, 128], F32)
            make_identity(nc, ident_f[:])
            ident_b = const.tile([128, 128], BF16)
            nc.vector.tensor_copy(ident_b[:], ident_f[:])

            zsc = const.tile([128, 128], F32)
            nc.vector.memset(zsc[:], 0.0)
            ones_f = const.tile([128, NKC], F32)
            nc.vector.memset(ones_f[:], 1.0)

            qT = big.tile([C, N], F32R)            # q^T  [c, token]
            kT = big.tile([C, NPAD], F32R)         # k^T  [c, token], zero pad
            vT = big.tile([128, N], BF16)          # v''^T on partitions 64-127
            v_nat = big.tile([128, NKC, C + 1], BF16)  # [tok%128, chunk, e|1]

            nc.vector.tensor_copy(kT[:, N:NPAD], zsc[0:C, 0:NPAD - N])
            nc.vector.tensor_copy(v_nat[:, :, C], ones_f[:])

            with tc.tile_pool(name="ps2", bufs=2, space="PSUM") as ps2, \
                 tc.tile_pool(name="psSa", bufs=1, space="PSUM") as psSa, \
                 tc.tile_pool(name="psSb", bufs=1, space="PSUM") as psSb, \
                 tc.tile_pool(name="psC", bufs=1, space="PSUM") as psC, \
                 tc.tile_pool(name="sbP", bufs=2 * NGRP) as sbP, \
                 tc.tile_pool(name="sbO", bufs=4) as sbO, \
                 tc.tile_pool(name="sbI", bufs=4) as sbI:

                # PE warmup: set pe_busy_start early so the clock ramps to
                # full rate during the image DMA wait
                ptw = ps2.tile([128, NQ], F32, name="ptw", tag="cv")
                for _ in range(16):
                    nc.tensor.matmul(ptw[0:128, 0:128], wtile[:, 0:128],
                                     wtile[:, 0:128], start=True, stop=True)

                # ---- incremental emitters ----------------------------------
                st = {"kv": 0, "vn": 0, "pctx": None}

                def conv_matmuls(pdst, lhsT, ct, mwid, slots=range(5)):
                    r0 = ct * 8
                    first = None
                    for i in slots:
                        s = SLOT_ORDER[i]
                        if s < 3:
                            rhs = xpT[:, r0:r0 + 8, s:s + W]
                            lh = lhsT[:, s, 0:mwid]
                        elif s == 3:
                            rhs = x3[:, r0 + 2:r0 + 10, 0:W]
                            lh = lhsT[:, s, 0:mwid]
                        else:
                            rhs = xpT[0:C, r0 + 2:r0 + 10, 2:2 + W]
                            lh = lhsT[0:C, s, 0:mwid]
                        mm = nc.tensor.matmul(pdst[:], lh, rhs,
                                              start=(i == 0), stop=(i == 4))
                        if first is None:
                            first = mm
                    return first

                def emit_vt_add(pkv, ct):
                    nc.vector.tensor_scalar_add(
                        vT[C:128, ct * NQ:(ct + 1) * NQ], pkv[C:128, :],
                        bqb1[C:128, 1:2])

                def emit_kv(upto):
                    while st["kv"] <= min(upto, NQT - 1):
                        ct = st["kv"]
                        pkv = ps2.tile([128, NQ], F32, name="pkv", tag="cv")
                        first = conv_matmuls(pkv, st_kv, ct, 128)
                        if ct == 1 and st.get("qk_dep") is not None:
                            # keep the tile scheduler from hoisting kv-conv 1
                            # ahead of QK group 0 on the PE (its CoreSim DMA
                            # model is optimistic about the x chunk arrival)
                            add_dep_helper(first.ins, st["qk_dep"].ins,
                                           reason="kv conv after QK group")
                        nc.vector.tensor_copy(kT[:, ct * NQ:(ct + 1) * NQ],
                                              pkv[0:C, :])
                        emit_vt_add(pkv, ct)
                        st["kv"] += 1

                def emit_vnat():
                    # v_nat transposes, batched 4 chunks per PSUM tile /
                    # copy to amortize the DVE PSUM-access overhead
                    top = st["kv"] * NQ
                    while st["vn"] < NKC:
                        kc0 = st["vn"]
                        nb = min(4, NKC - kc0)
                        end = kc0 + nb - 1
                        cw_last = min(KC, N - end * KC)
                        if end * KC + cw_last > top:
                            break
                        tp = psC.tile([128, 4, C], BF16, name="tpv",
                                      tag="ctx")
                        for j in range(nb):
                            kc = kc0 + j
                            cw = min(KC, N - kc * KC)
                            nc.tensor.transpose(
                                tp[0:cw, j, :],
                                vT[C:128, kc * KC:kc * KC + cw],
                                ident_b[C:128, C:128])
                        cw = min(KC, N - (kc0 + nb - 1) * KC)
                        if nb == 4 and cw == KC:
                            nc.vector.tensor_copy(
                                v_nat[:, kc0:kc0 + nb, 0:C], tp[:, 0:nb, :])
                        else:
                            for j in range(nb):
                                kc = kc0 + j
                                cw = min(KC, N - kc * KC)
                                nc.vector.tensor_copy(
                                    v_nat[0:cw, kc, 0:C], tp[0:cw, j, :])
                        st["vn"] += nb

                def emit_qconv_slots(pq, qt, slots):
                    r0 = qt * 8
                    for i in slots:
                        s = SLOT_ORDER[i]
                        if s < 3:
                            rhs = xpT[:, r0:r0 + 8, s:s + W]
                            lh = st_q[:, s, :]
                        elif s == 3:
                            rhs = x3[:, r0 + 2:r0 + 10, 0:W]
                            lh = st_q[:, s, :]
                        else:
                            rhs = xpT[0:C, r0 + 2:r0 + 10, 2:2 + W]
                            lh = st_q[0:C, s, :]
                        nc.tensor.matmul(pq[:], lh, rhs,
                                         start=(i == 0), stop=(i == 4))

                def emit_qcopy(pq, qt):
                    if qt == 0:
                        # ACT is idle before the first exp; DVE is busy with
                        # lhsT builds
                        nc.scalar.add(qT[:, qt * NQ:(qt + 1) * NQ], pq[:],
                                      bqb1[0:C, 0:1])
                    else:
                        nc.vector.tensor_scalar_add(
                            qT[:, qt * NQ:(qt + 1) * NQ], pq[:], bqb1[0:C, 0:1])

                def emit_av_chunks(pT_tiles, s, pctx, chunks):
                    # chunk-matmuls of one q-subtile into accumulator pctx
                    s0 = s * 128
                    sw = min(128, NQ - s0)
                    for kc in chunks:
                        g = _grp_of(kc)
                        j = kc - GRP_OFF[g]
                        cw = 64 if kc == NKC - 1 else 128
                        nc.tensor.matmul(
                            pctx[0:sw, :],
                            pT_tiles[g][0:cw, j, s0:s0 + sw],
                            v_nat[0:cw, kc, :],
                            start=(kc == 0), stop=(kc == NKC - 1))

                def emit_av_batch(pT_tiles, s, half):
                    # 25 chunk-matmuls of one q-subtile, split in two halves;
                    # one pending psum group at a time (zero-region rule)
                    if half == 0:
                        st["pctx"] = psC.tile([128, C + 1], F32,
                                              name="pctx", tag="ctx")
                    chunks = range(0, 13) if half == 0 else range(13, NKC)
                    emit_av_chunks(pT_tiles, s, st["pctx"], chunks)

                def emit_norm_sub(pctx, qt, s):
                    # normalize into the tile's gathered [128, 4, E] buffer;
                    # sub 3 triggers the single per-tile store DMA
                    s0 = s * 128
                    sw = min(128, NQ - s0)
                    inv = sbI.tile([128, 1], F32, name="inv", tag="inv")
                    nc.vector.reciprocal(inv[0:sw, :], pctx[0:sw, C:C + 1])
                    if s == 0:
                        st["osb4"] = sbO.tile([128, 4, E], F32, name="osb4",
                                              tag="out")
                    osb4 = st["osb4"]
                    nc.vector.tensor_scalar_mul(
                        osb4[0:sw, s, :], pctx[0:sw, 0:C], inv[0:sw, 0:1])
                    if s == 3:
                        nc.sync.dma_start(
                            out_d[qt, :, :].rearrange("(s p) e -> p s e", p=128),
                            osb4[:, :, :])

                def flush_prev(prev, g):
                    # AV batches one group later than minimal so the psC WAR
                    # (sub start vs previous norm) never stalls the PE
                    if prev is None or g < 1 or g > 8:
                        return
                    qt_prev, pT_tiles = prev
                    emit_av_batch(pT_tiles, (g - 1) // 2, (g - 1) % 2)
                    if (g - 1) % 2 == 1:
                        emit_norm_sub(st["pctx"], qt_prev, (g - 1) // 2)

                # ---- lead-in: q-conv(0) + kv(0) ----------------------------
                # PE order tuned for the kT critical path: xpT-only slots of
                # both convs first, then kv's x3 slot (so the kT copy can
                # start ASAP), then q's x3 slot + bias copy. vT add deferred
                # past QK g0 (engine-counter waits would otherwise put it on
                # QK's critical path).
                tap_sched = {0: (0,), 1: (1,), 2: (2,), 3: (3,), 4: (4,)}
                if level >= 2:
                    pq = ps2.tile([C, NQ], F32, name="pq", tag="cv")
                    emit_qconv_slots(pq, 0, range(4))
                    pkv0 = ps2.tile([128, NQ], F32, name="pkv", tag="cv")
                    conv_matmuls(pkv0, st_kv, 0, 128, slots=range(4))
                    conv_matmuls(pkv0, st_kv, 0, 128, slots=(4,))
                    nc.vector.tensor_copy(kT[:, 0:NQ], pkv0[0:C, :])
                    emit_qconv_slots(pq, 0, (4,))
                    emit_qcopy(pq, 0)
                    st["kv"] = 1
                    st["pkv0"] = pkv0

                prev = None
                last = NQT - 1
                for qt in range(NQT if level >= 5 else 0):
                    q0 = qt * NQ
                    pq_next = None
                    pT_tiles = []
                    pc6 = [None, None, None, None]  # last-tile accumulators
                    if qt == last and level >= 6:
                        pc6[0] = ps2.tile([128, C + 1], F32, name="pc6a",
                                          tag="cv")
                        pc6[1] = ps2.tile([128, C + 1], F32, name="pc6b",
                                          tag="cv")
                    for g in range(NGRP):
                        gsz = GRP_SIZES[g]
                        pool = psSa if g % 2 == 0 else psSb
                        ps_s = pool.tile([128, gsz, 512], F32, name="ps_s",
                                         tag="sa" if g % 2 == 0 else "sb")
                        for j in range(gsz):
                            kc = GRP_OFF[g] + j
                            qk_mm = nc.tensor.matmul(
                                ps_s[:, j, 0:NQ],
                                kT[:, kc * KC:(kc + 1) * KC],
                                qT[:, q0:q0 + NQ],
                                start=True, stop=True)
                        st["qk_dep"] = qk_mm
                        flush_prev(prev, g)
                        if qt == last and level >= 6:
                            if g == 8:
                                # tile-5 AV fully flushed; psC free for sub 3
                                pc6[3] = psC.tile([128, C + 1], F32,
                                                  name="pc6d", tag="ctx")
                            if g >= 1:
                                lo = GRP_OFF[g - 1]
                                hi = lo + GRP_SIZES[g - 1]
                                emit_av_chunks(pT_tiles, 0, pc6[0],
                                               range(lo, hi))
                                emit_av_chunks(pT_tiles, 1, pc6[1],
                                               range(lo, hi))
                            if g == NGRP - 1:
                                # catch-up: subs 2/3 over groups 0..8
                                hi = GRP_OFF[g]
                                emit_av_chunks(pT_tiles, 2, pc6[2],
                                               range(0, hi))
                                emit_av_chunks(pT_tiles, 3, pc6[3],
                                               range(0, hi))
                        if qt == 0:
                            # kv-conv tiles + v_nat stream in under tile 0
                            if g == 0:
                                emit_vt_add(st["pkv0"], 0)
                            if g + 1 < NGRP:
                                emit_kv(KV_NEED[g + 1])
                                emit_vnat()
                            elif qt + 1 < NQT:
                                emit_vnat()
                                pq_next = ps2.tile([C, NQ], F32,
                                                   name="pq", tag="cv")
                                emit_qconv_slots(pq_next, 1, range(5))
                                emit_qcopy(pq_next, 1)
                        elif qt + 1 < NQT:
                            if g in tap_sched:
                                if pq_next is None:
                                    pq_next = ps2.tile([C, NQ], F32,
                                                       name="pq", tag="cv")
                                emit_qconv_slots(pq_next, qt + 1, tap_sched[g])
                            if g == 5:
                                emit_qcopy(pq_next, qt + 1)
                        pTt = sbP.tile([128, 3, NQ], BF16, name="pTt", tag="p")
                        nc.scalar.activation(
                            pTt[:, 0:gsz, :], ps_s[:, 0:gsz, 0:NQ],
                            AF.Exp, scale=8.0)
                        pT_tiles.append(pTt)
                        if qt == last and level >= 6 and g == 8:
                            # psSa's last score group read; slot free for sub 2
                            pc6[2] = psSa.tile([128, C + 1], F32,
                                               name="pc6c", tag="sa")
                    if level >= 6:
                        if qt < last:
                            prev = (qt, pT_tiles)
                        else:
                            # tail: last group's chunks for all subs, then
                            # norm + store per sub
                            lo = GRP_OFF[NGRP - 1]
                            hi = lo + GRP_SIZES[NGRP - 1]
                            for s in range(4):
                                emit_av_chunks(pT_tiles, s, pc6[s],
                                               range(lo, hi))
                            for s in range(4):
                                emit_norm_sub(pc6[s], qt, s)

    nc.compile()
    return nc


def _get_nc():
    if "nc" not in _CACHE:
        _CACHE["nc"] = _build()
    return _CACHE["nc"]


def kernel(x, wq, bq, wk, bk, wv, bv, Wp, bp):
    from concourse.bass_utils import run_bass_kernel_spmd

    nc = _get_nc()
    x = np.asarray(x, dtype=np.float32)
    stq, stkv, bqb1 = _prep_weights(wq, wk, wv, bq, bv, Wp, bp)
    shared = {"stq": stq, "stkv": stkv, "bqb1": bqb1}
    in_maps = []
    for i in range(NCORES):
        xp, x3 = _prep_x(x[i])
        in_maps.append(dict(shared, x=xp, x3=x3))
    res = run_bass_kernel_spmd(nc, in_maps, core_ids=list(range(NCORES)))
    out = np.stack([
        np.asarray(res.results[i]["out"])
        .reshape(NQT, 4 * 128, E)[:, 0:NQ, :].reshape(H, W, E)
        for i in range(NCORES)
    ])
    return out
